# revision 1
# baseline (speedup 1.0000x reference)
"""Trainium2 Bass kernel for nn_DGSL_3453153706625 (gnn_message_passing).

Strategy (data-parallel over graphs, 8 graphs per core):
  * Only the nodes referenced by gather_idx matter for the micro GCN output
    (<=250 unique per graph), and only the final timestep of the Mamba scan
    feeds the output.  Per graph we build 256 dst "slots" (2 windows of 128)
    and extract the edges whose dst is in the slot set (+1 self edge/slot).
  * Host does index/layout prep only: per-core transposed x-slabs (subgraph
    feature extraction), per-edge src-degree weight lists (padded), dst-local
    indices, edge weights.  All FLOPs run on device:
      deg = rowsum(list); dinv = sqrt(1/deg); edge scale = dinv_src*ew
      h = x @ W (fp32r matmuls), scale fused into PSUM evacuation (ACT),
      scatter-to-slot via on-device is_equal selection matmuls,
      dst dinv fused into aggregation evacuation, masked mean via small
      G matmuls -> seq^T, Mamba last-state algebra (suffix sum via
      triangular matmul, exp, B.C_last dots, weighted t-reduction),
      macro GCN + mean pool, final MLP.  Output [2H, B/core]^T per core.
"""

import math
from dataclasses import dataclass

import numpy as np

import concourse.bass as bass
import concourse.tile as tile
from concourse import bacc
from concourse import mybir
from concourse import bass_utils

F32 = mybir.dt.float32
F32R = mybir.dt.float32r
BF16 = mybir.dt.bfloat16


@dataclass
class Cfg:
    n_cores: int = 8
    gpc: int = 8            # graphs per core
    T: int = 50             # seq len
    NG: int = 5             # nodes per group
    n_micro: int = 131072
    e_micro: int = 1048576
    n_macro: int = 6400
    e_macro: int = 51200
    npm: int = 100          # nodes per macro graph
    in_dim: int = 384
    h: int = 256
    s: int = 64
    chunk_tiles: int = 16   # x-slab DMA chunk, in 128-col tiles
    # dtype knobs
    slab_bf16: bool = False  # x-slabs + Wg in bf16 (halves DMA)
    use_f32r: bool = True    # fp32r for the big matmuls
    s_conv: bool = False     # is_equal writes agg dtype directly

    @property
    def B(self):
        return self.n_cores * self.gpc

    @property
    def KC(self):
        return self.in_dim // 128

    @property
    def HC(self):
        return self.h // 128


REAL = Cfg()


# ---------------------------------------------------------------- host prep

def _csr_by_dst(dst, ew, n_nodes):
    order = np.argsort(dst, kind="stable")
    counts = np.bincount(dst, minlength=n_nodes).astype(np.int64)
    offs = np.concatenate([[0], np.cumsum(counts)])[:-1]
    return counts, offs, ew[order]


def _deg_lists(node_ids, counts, offs, csr_ew, W):
    """[M, W] padded incoming-edge-weight lists with the +1.0 self entry."""
    node_ids = np.asarray(node_ids, dtype=np.int64)
    M = len(node_ids)
    cnts = counts[node_ids]
    pos = offs[node_ids][:, None] + np.arange(W)[None, :]
    pos = np.minimum(pos, max(len(csr_ew) - 1, 0))
    valid = np.arange(W)[None, :] < cnts[:, None]
    out = np.where(valid, csr_ew[pos], 0.0).astype(np.float32)
    out[np.arange(M), cnts] = 1.0  # self-loop +1
    return out


def _tile_layout_rows(arr_2d, tiles, width):
    """[tiles*128, W] -> [128, tiles*W] partition-line layout."""
    a = arr_2d.reshape(tiles, 128, width).transpose(1, 0, 2)
    return np.ascontiguousarray(a.reshape(128, tiles * width))


def _col_layout(arr_1d, tiles):
    """[tiles*128] -> [128, tiles]."""
    return np.ascontiguousarray(arr_1d.reshape(tiles, 128).T)


def _prep_branch(x, src_all, dst_all, ew_all, n_nodes, slot_nodes, cfg,
                 n_windows_per_graph, gmap=None):
    """Shared micro/macro edge-extraction.

    slot_nodes: list of B arrays (sorted node ids per graph's slots).
    Returns dict with per-core slabs and shared meta.
    """
    B, gpc, ncores = cfg.B, cfg.gpc, cfg.n_cores
    nwg = n_windows_per_graph
    counts, offs, csr_ew = _csr_by_dst(dst_all, ew_all, n_nodes)
    W = int(counts.max()) + 1
    W = int(math.ceil(W / 4) * 4)

    # node -> (graph, local) multimap
    n_g = np.array([len(u) for u in slot_nodes])
    cat_nodes = np.concatenate(slot_nodes)
    cat_graph = np.repeat(np.arange(B), n_g)
    cat_local = np.concatenate([np.arange(n) for n in n_g])
    ordn = np.argsort(cat_nodes, kind="stable")
    snodes = cat_nodes[ordn]

    le = np.searchsorted(snodes, dst_all, "left")
    ri = np.searchsorted(snodes, dst_all, "right")
    cnt = ri - le
    sel = np.flatnonzero(cnt)
    c = cnt[sel]
    rep = np.repeat(sel, c)
    startrep = np.repeat(le[sel], c)
    within = np.arange(int(c.sum())) - np.repeat(np.cumsum(c) - c, c)
    matchpos = ordn[startrep + within]

    e_graph = np.concatenate([cat_graph[matchpos], cat_graph])
    e_local = np.concatenate([cat_local[matchpos], cat_local])
    e_src = np.concatenate([src_all[rep], cat_nodes])
    e_ew = np.concatenate([ew_all[rep], np.ones(len(cat_nodes), np.float32)])

    e_win = e_local // 128
    e_dl = (e_local % 128).astype(np.float32)
    if gmap is None:
        # balance: assign graphs to (core, gpos) so that same-gpos graphs
        # across cores have similar edge counts (cuts the cross-core max
        # padding); sorted-rank round-robin.
        counts_g = np.bincount(e_graph, minlength=B)
        rank = np.argsort(-counts_g, kind="stable")
        gmap = np.empty(B, np.int64)
        for r, g in enumerate(rank):
            gmap[g] = (r % ncores) * gpc + (r // ncores)
    e_slot = gmap[e_graph]
    key = e_slot * nwg + e_win
    counts_gw = np.bincount(key, minlength=B * nwg)
    tiles_gw = np.ceil(counts_gw / 128).astype(np.int64)
    tiles_gw = np.maximum(tiles_gw, 1)
    Tpos = tiles_gw.reshape(ncores, gpc * nwg).max(axis=0)  # [gpc*nwg]
    pos_off = np.concatenate([[0], np.cumsum(Tpos * 128)])
    EM = int(pos_off[-1])

    orde = np.argsort(key, kind="stable")
    segoff = np.concatenate([[0], np.cumsum(counts_gw)])

    srcs = np.zeros((ncores, EM), np.int64)
    ews = np.zeros((ncores, EM), np.float32)
    dloc = np.full((ncores, EM), -1.0, np.float32)
    for g in range(B):
        slot = int(gmap[g])
        core, gpos = slot // gpc, slot % gpc
        for w in range(nwg):
            k = slot * nwg + w
            ck = int(counts_gw[k])
            sl = orde[segoff[k]:segoff[k] + ck]
            o = int(pos_off[gpos * nwg + w])
            srcs[core, o:o + ck] = e_src[sl]
            ews[core, o:o + ck] = e_ew[sl]
            dloc[core, o:o + ck] = e_dl[sl]

    tiles = EM // 128
    slab_dtype = np.dtype("bfloat16") if cfg.slab_bf16 else np.float32
    per_core = []
    for core in range(ncores):
        xs = x[srcs[core]].astype(np.float32).T  # [in_dim, EM]
        xs = np.ascontiguousarray(xs.reshape(x.shape[1] // 128, 128, EM))
        if cfg.slab_bf16:
            import ml_dtypes  # noqa
            xs = xs.astype(ml_dtypes.bfloat16)
        deg = _deg_lists(srcs[core], counts, offs, csr_ew, W)
        per_core.append(dict(
            xs=xs,
            deg=_tile_layout_rows(deg, tiles, W),
            dl=_col_layout(dloc[core], tiles),
            ew=_col_layout(ews[core], tiles),
        ))

    # dst-slot degree lists: [ncores][128, nW*W]
    nW = gpc * nwg
    inv = np.empty(B, np.int64)
    inv[gmap] = np.arange(B)
    for core in range(ncores):
        slot_ids = np.zeros((nW, 128), np.int64)
        for gpos in range(gpc):
            g = int(inv[core * gpc + gpos])
            u = slot_nodes[g]
            for w in range(nwg):
                seg = u[w * 128:(w + 1) * 128]
                slot_ids[gpos * nwg + w, :len(seg)] = seg
        degd = _deg_lists(slot_ids.ravel(), counts, offs, csr_ew, W)
        per_core[core]["degd"] = _tile_layout_rows(degd, nW, W)

    return dict(per_core=per_core, Tpos=Tpos, EM=EM, W=W, dtype=slab_dtype,
                gmap=gmap)


def prep_host(inputs, cfg):
    gi = np.asarray(inputs["gather_idx"]).astype(np.int64)  # [B, T, NG]
    mask = np.asarray(inputs["mask"]).astype(np.float32)    # [B, T]
    B, gpc, T, NG = cfg.B, cfg.gpc, cfg.T, cfg.NG

    uniq = [np.unique(gi[g]) for g in range(B)]
    for u in uniq:
        assert len(u) <= 256
    mic = _prep_branch(
        np.asarray(inputs["micro_x"]),
        np.asarray(inputs["micro_ei"][0]).astype(np.int64),
        np.asarray(inputs["micro_ei"][1]).astype(np.int64),
        np.asarray(inputs["micro_ew"]).astype(np.float32),
        cfg.n_micro, uniq, cfg, 2)

    gmap = mic["gmap"]
    mac_slots = [np.arange(g * cfg.npm, (g + 1) * cfg.npm) for g in range(B)]
    mac = _prep_branch(
        np.asarray(inputs["macro_x"]),
        np.asarray(inputs["macro_ei"][0]).astype(np.int64),
        np.asarray(inputs["macro_ei"][1]).astype(np.int64),
        np.asarray(inputs["macro_ew"]).astype(np.float32),
        cfg.n_macro, mac_slots, cfg, 1, gmap=gmap)

    # G slab (mask/NG at (slot, t)) and mask rows, per core
    NWm = gpc * 2
    Gall = np.zeros((cfg.n_cores, NWm, 128, T), np.float32)
    g_idx = np.repeat(np.arange(B), T * NG)
    t_idx = np.tile(np.repeat(np.arange(T), NG), B)
    n_idx = gi.ravel()
    loc = np.concatenate(
        [np.searchsorted(uniq[g], gi[g].ravel()) for g in range(B)])
    slot_i = gmap[g_idx]
    core_i = slot_i // gpc
    win_i = (slot_i % gpc) * 2 + loc // 128
    row_i = loc % 128
    val = mask[g_idx, t_idx] / NG
    np.add.at(Gall, (core_i, win_i, row_i, t_idx), val)
    del n_idx

    # consts
    iotaF = np.tile(np.arange(128, dtype=np.float32)[None, :], (128, 1))
    T1 = np.zeros((128, T), np.float32)
    tt = np.arange(T)
    T1[:T, :] = (tt[:, None] > tt[None, :]).astype(np.float32)  # [tau, t]
    ones1 = np.ones((1, 128), np.float32)
    poolmat = np.zeros((128, gpc * gpc), np.float32)
    for g in range(gpc):
        poolmat[:cfg.npm, g * gpc + g] = 1.0 / cfg.npm

    wdt = np.asarray(inputs["W_dtBC"]).astype(np.float32)  # [h, 1+2s]
    s = cfg.s
    wdt_perm = np.concatenate(
        [wdt[:, 1 + s:1 + 2 * s], wdt[:, 1:1 + s], wdt[:, :1]], axis=1)

    f32 = np.float32
    shared = {
        "Wg_mic": np.ascontiguousarray(np.asarray(inputs["Wg_micro"]).astype(
            mic["dtype"] if cfg.slab_bf16 else f32)),
        "Wg_mac": np.ascontiguousarray(np.asarray(inputs["Wg_macro"]).astype(
            mac["dtype"] if cfg.slab_bf16 else f32)),
        "bgm_row": np.asarray(inputs["bg_micro"]).astype(f32).reshape(1, -1),
        "bgcT": np.asarray(inputs["bg_macro"]).astype(f32).reshape(-1, 1),
        "W_in": np.asarray(inputs["W_in"]).astype(f32),
        "WdtP": np.ascontiguousarray(wdt_perm),
        "dtb": np.asarray(inputs["dt_bias"]).astype(f32).reshape(1, 1),
        "A_logT": np.asarray(inputs["A_log"]).astype(f32).reshape(-1, 1),
        "DpT": np.asarray(inputs["Dp"]).astype(f32).reshape(-1, 1),
        "W_out": np.asarray(inputs["W_out"]).astype(f32),
        "W1": np.asarray(inputs["W1"]).astype(f32),
        "b1T": np.asarray(inputs["b1"]).astype(f32).reshape(-1, 1),
        "W2": np.asarray(inputs["W2"]).astype(f32),
        "b2T": np.asarray(inputs["b2"]).astype(f32).reshape(-1, 1),
        "iotaF": iotaF, "T1": T1, "ones1": ones1, "poolmat": poolmat,
    }

    inv_g = np.empty(B, np.int64)
    inv_g[gmap] = np.arange(B)
    in_maps = []
    for core in range(cfg.n_cores):
        m = dict(shared)
        pc, qc = mic["per_core"][core], mac["per_core"][core]
        m.update({
            "xs_mic": pc["xs"], "deg_mic": pc["deg"], "dl_mic": pc["dl"],
            "ew_mic": pc["ew"], "degd_mic": pc["degd"],
            "xs_mac": qc["xs"], "deg_mac": qc["deg"], "dl_mac": qc["dl"],
            "ew_mac": qc["ew"], "degd_mac": qc["degd"],
            "Gslab": np.ascontiguousarray(
                Gall[core].transpose(1, 0, 2).reshape(128, NWm * T)),
            "maskrow": np.ascontiguousarray(
                mask[inv_g[core * gpc:(core + 1) * gpc]].reshape(
                    1, gpc * T)),
        })
        in_maps.append(m)

    meta = dict(
        Tpos_mic=mic["Tpos"], EM=mic["EM"], Wmic=mic["W"],
        Tpos_mac=mac["Tpos"], EA=mac["EM"], Wmac=mac["W"],
        gmap=gmap,
    )
    return in_maps, meta


# ---------------------------------------------------------------- device

def build_nc(cfg, meta):
    T, gpc, h, s = cfg.T, cfg.gpc, cfg.h, cfg.s
    KC, HC = cfg.KC, cfg.HC
    DC = 1 + 2 * s
    assert 2 * s <= 128 and T <= 128 and gpc * T <= 512
    EM, EA = meta["EM"], meta["EA"]
    Wmic, Wmac = meta["Wmic"], meta["Wmac"]
    NWm, NWa = gpc * 2, gpc
    TM, TA = EM // 128, EA // 128
    if cfg.slab_bf16:
        sdt = BF16        # x-slab / Wg dtype
    elif cfg.use_f32r:
        sdt = F32R
    else:
        sdt = F32
    # aggregation operand dtype matches the slab dtype so the agg
    # matmuls run at 1 cyc/row (Bacc's generate_event_semaphores legalizes
    # the multi-wait producers)
    adt = sdt

    nc = bacc.Bacc("TRN2")
    D = {}
    def din(name, shape, dt=F32):
        D[name] = nc.dram_tensor(name, list(shape), dt, kind="ExternalInput")
        return D[name]

    din("xs_mic", (KC, 128, EM), sdt)
    din("deg_mic", (128, TM * Wmic))
    din("dl_mic", (128, TM))
    din("ew_mic", (128, TM))
    din("degd_mic", (128, NWm * Wmic))
    din("xs_mac", (KC, 128, EA), sdt)
    din("deg_mac", (128, TA * Wmac))
    din("dl_mac", (128, TA))
    din("ew_mac", (128, TA))
    din("degd_mac", (128, NWa * Wmac))
    din("Gslab", (128, NWm * T))
    din("maskrow", (1, gpc * T))
    din("Wg_mic", (cfg.in_dim, h), sdt)
    din("Wg_mac", (cfg.in_dim, h), sdt)
    din("bgm_row", (1, h))
    din("bgcT", (h, 1))
    din("W_in", (h, 2 * h))
    din("WdtP", (h, DC))
    din("dtb", (1, 1))
    din("A_logT", (h, 1))
    din("DpT", (h, 1))
    din("W_out", (h, h))
    din("W1", (2 * h, h))
    din("b1T", (h, 1))
    din("W2", (h, 2 * h))
    din("b2T", (2 * h, 1))
    din("iotaF", (128, 128))
    din("T1", (128, T))
    din("ones1", (1, 128))
    din("poolmat", (128, gpc * gpc))
    outT = nc.dram_tensor("outT", [2 * h, gpc], F32, kind="ExternalOutput")
    dt_scratch = nc.dram_tensor("dt_scratch", [gpc * T], F32, kind="Internal")
    sdt_scratch = nc.dram_tensor("sdt_scratch", [gpc * T], F32,
                                 kind="Internal")

    with tile.TileContext(nc) as tc:
        with (
            tc.tile_pool(name="const", bufs=1) as cp,
            tc.tile_pool(name="xs", bufs=2) as xp,
            tc.tile_pool(name="degs", bufs=2) as dp,
            tc.tile_pool(name="work", bufs=8) as wp,
            tc.tile_pool(name="ph", bufs=3, space="PSUM") as ph,
            tc.tile_pool(name="pagg", bufs=3, space="PSUM") as pagg,
            tc.tile_pool(name="ptail", bufs=2, space="PSUM") as pt,
        ):
            def pe_touch(ap_col):
                """Dummy weight-load so PE's vector clock absorbs the DMA
                wait of a fp32r operand before its real (1-wait-budget)
                matmul.  No PSUM output, single LW struct, single wait."""
                nc.tensor.ldweights(ap_col.bitcast(BF16))
            def load_const(name, funnel=None):
                src = D[name]
                t = cp.tile(list(src.shape), src.dtype, tag=name)
                nc.sync.dma_start(t[:], src[:])
                if funnel == "act":
                    t2 = cp.tile(list(src.shape), src.dtype, tag=name + "_f")
                    nc.scalar.copy(t2[:], t[:])
                    return t2
                if funnel == "dve":
                    t2 = cp.tile(list(src.shape), src.dtype, tag=name + "_f")
                    nc.vector.tensor_copy(t2[:], t[:])
                    return t2
                return t

            def load_mat_chunks(name, k, n, dt=F32, funnel=None):
                """[k, n] dram -> SBUF [128, (k//128)*n], chunk kc at
                cols [kc*n:(kc+1)*n].  Single DMA."""
                kc_n = k // 128
                t = cp.tile([128, kc_n * n], dt, tag=name)
                nc.sync.dma_start(
                    t[:].rearrange("p (c n) -> p c n", c=kc_n),
                    D[name][:].rearrange("(c p) n -> p c n", p=128))
                if funnel == "act":
                    t2 = cp.tile([128, kc_n * n], dt, tag=name + "_f")
                    nc.scalar.copy(t2[:], t[:])
                    return t2
                if funnel == "dve":
                    t2 = cp.tile([128, kc_n * n], dt, tag=name + "_f")
                    nc.vector.tensor_copy(t2[:], t[:])
                    return t2
                return t

            wgmic = load_mat_chunks("Wg_mic", cfg.in_dim, h, sdt)
            wgmac = load_mat_chunks("Wg_mac", cfg.in_dim, h, sdt)
            for kc in range(KC):
                pe_touch(wgmic[:, kc * h:kc * h + 1])
                pe_touch(wgmac[:, kc * h:kc * h + 1])
            iota = load_const("iotaF")

            def act_funnel(t, tag):
                t2 = cp.tile(list(t.shape), t.dtype, tag=tag)
                nc.scalar.copy(t2[:], t[:])
                return t2

            def gcn_branch(tag, xs_d, deg_d, dl_d, ew_d, degd_d, Tpos, nwin,
                           Wd, ntiles, wg_sb, nwg, co_steps=None):
                # dst dinv per window
                degd_sb = cp.tile([128, nwin * Wd], F32, tag=f"degd{tag}")
                nc.sync.dma_start(degd_sb[:], degd_d[:])
                dsum = cp.tile([128, nwin], F32, tag=f"dsum{tag}")
                nc.vector.tensor_reduce(
                    dsum[:], degd_sb[:].rearrange("p (w d) -> p w d", d=Wd),
                    axis=mybir.AxisListType.X, op=mybir.AluOpType.add)
                nc.vector.reciprocal(dsum[:], dsum[:])
                dinvd = cp.tile([128, nwin], F32, tag=f"dinvd{tag}")
                nc.scalar.sqrt(dinvd[:], dsum[:])

                dl_sb = cp.tile([128, ntiles], F32, tag=f"dl{tag}")
                nc.sync.dma_start(dl_sb[:], dl_d[:])
                ew_sb = cp.tile([128, ntiles], F32, tag=f"ew{tag}")
                nc.sync.dma_start(ew_sb[:], ew_d[:])

                gcnw = cp.tile([128, nwin * h], F32, tag=f"gcnw{tag}")

                # tile -> window map
                win_of, idx_in, len_of = [], [], []
                for p, tp in enumerate(Tpos):
                    for i in range(int(tp)):
                        win_of.append(p)
                        idx_in.append(i)
                        len_of.append(int(tp))

                CT = cfg.chunk_tiles
                agg = None
                nch = (ntiles + CT - 1) // CT
                co_done = 0
                for c0 in range(0, ntiles, CT):
                    ct = min(CT, ntiles - c0)
                    xts = []
                    for kc in range(KC):
                        xt = xp.tile([128, CT * 128], sdt, tag=f"x{kc}")
                        nc.sync.dma_start(
                            xt[:, :ct * 128],
                            xs_d[kc, :, c0 * 128:(c0 + ct) * 128])
                        pe_touch(xt[:, 0:1])
                        xts.append(xt)
                    degt = dp.tile([128, CT * Wd], F32, tag="degc")
                    nc.sync.dma_start(
                        degt[:, :ct * Wd],
                        deg_d[:, c0 * Wd:(c0 + ct) * Wd])
                    scal = dp.tile([128, CT], F32, tag="scalc")
                    nc.vector.tensor_reduce(
                        scal[:, :ct],
                        degt[:, :ct * Wd].rearrange("p (t d) -> p t d", d=Wd),
                        axis=mybir.AxisListType.X, op=mybir.AluOpType.add)
                    nc.vector.reciprocal(scal[:, :ct], scal[:, :ct])
                    nc.scalar.sqrt(scal[:, :ct], scal[:, :ct])
                    nc.vector.tensor_tensor(
                        out=scal[:, :ct], in0=scal[:, :ct],
                        in1=ew_sb[:, c0:c0 + ct], op=mybir.AluOpType.mult)

                    for i in range(ct):
                        ti = c0 + i
                        S = wp.tile([128, 128], adt, tag="S0")
                        nc.vector.tensor_tensor(
                            out=S[:], in0=iota[:],
                            in1=dl_sb[:, ti:ti + 1].to_broadcast([128, 128]),
                            op=mybir.AluOpType.is_equal)
                        hp_t = ph.tile([128, h], F32, tag="hp")
                        for kc in range(KC):
                            nc.tensor.matmul(
                                hp_t[:],
                                lhsT=xts[kc][:, i * 128:(i + 1) * 128],
                                rhs=wg_sb[:, kc * h:(kc + 1) * h],
                                start=(kc == 0), stop=(kc == KC - 1))
                        hs = wp.tile([128, h], adt, tag="hs0")
                        if ti % 2 == 0:
                            nc.scalar.mul(hs[:], hp_t[:], scal[:, i:i + 1])
                        else:
                            nc.vector.tensor_tensor(
                                out=hs[:], in0=hp_t[:],
                                in1=scal[:, i:i + 1].to_broadcast([128, h]),
                                op=mybir.AluOpType.mult)
                        if idx_in[ti] == 0:
                            agg = pagg.tile([128, h], F32, tag="agg")
                        nc.tensor.matmul(
                            agg[:], lhsT=S[:], rhs=hs[:],
                            start=(idx_in[ti] == 0),
                            stop=(idx_in[ti] == len_of[ti] - 1))
                        if idx_in[ti] == len_of[ti] - 1:
                            w = win_of[ti]
                            nc.scalar.mul(
                                gcnw[:, w * h:(w + 1) * h], agg[:],
                                dinvd[:, w:w + 1])
                    if co_steps is not None:
                        want = (len(co_steps) * (c0 // CT + 1)) // nch
                        while co_done < want:
                            co_steps[co_done]()
                            co_done += 1
                if co_steps is not None:
                    while co_done < len(co_steps):
                        co_steps[co_done]()
                        co_done += 1
                return gcnw

            gcn_mic = gcn_branch(
                "m", D["xs_mic"], D["deg_mic"], D["dl_mic"], D["ew_mic"],
                D["degd_mic"], meta["Tpos_mic"], NWm, Wmic, TM, wgmic, 2)

            t1c = load_const("T1")
            ones1 = load_const("ones1")
            poolm = load_const("poolmat")
            gsl = load_const("Gslab")
            mrow = load_const("maskrow")
            win_sb = load_mat_chunks("W_in", h, 2 * h)
            wdt_sb = load_mat_chunks("WdtP", h, DC)
            wout_sb = load_mat_chunks("W_out", h, h)
            w1_sb = load_mat_chunks("W1", 2 * h, h)
            w2_sb = load_mat_chunks("W2", h, 2 * h)
            bgm = load_const("bgm_row")
            bgc = load_mat_chunks("bgcT", h, 1)
            b1c = load_mat_chunks("b1T", h, 1)
            b2c = load_mat_chunks("b2T", 2 * h, 1)
            alog = load_mat_chunks("A_logT", h, 1)
            dpc = load_mat_chunks("DpT", h, 1)
            dtb = load_const("dtb")
            # ---- tail (seq^T + mamba), emitted as steps interleaved
            # into the macro branch's DMA-bound chunk loop
            GT = gpc * T
            seqT = cp.tile([128, HC * gpc * T], F32, tag="seqT")

            def seq_cc(cc):
                return seqT[:, cc * gpc * T:(cc + 1) * gpc * T]

            def step_seq(g):
                for cc in range(HC):
                    ps = pt.tile([128, T], F32, tag="tp")
                    nc.tensor.matmul(
                        ps[:], lhsT=bgm[0:1, cc * 128:(cc + 1) * 128],
                        rhs=mrow[0:1, g * T:(g + 1) * T],
                        start=True, stop=False)
                    for w in range(2):
                        wi = g * 2 + w
                        nc.tensor.matmul(
                            ps[:],
                            lhsT=gcn_mic[:, wi * h + cc * 128:
                                         wi * h + cc * 128 + 128],
                            rhs=gsl[:, wi * T:(wi + 1) * T],
                            start=False, stop=(w == 1))
                    nc.scalar.copy(
                        seqT[:, cc * gpc * T + g * T:
                             cc * gpc * T + (g + 1) * T], ps[:])

            def pe_tail(lhsT_list, rhs_list, n, tag="tp", mrows=128):
                p = pt.tile([128, n], F32, tag=tag)
                kn = len(lhsT_list)
                for i, (l, r) in enumerate(zip(lhsT_list, rhs_list)):
                    nc.tensor.matmul(p[:mrows, :], lhsT=l, rhs=r,
                                     start=(i == 0), stop=(i == kn - 1))
                return p

            xzT = cp.tile([128, 4 * GT], F32, tag="xzT")
            dbc0 = cp.tile([128, GT], F32, tag="dbc0")
            dtsp = cp.tile([1, GT], F32, tag="dtsp")
            dt2 = cp.tile([128, gpc], F32, tag="dt2")
            sdt2 = cp.tile([128, gpc], F32, tag="sdt2")
            sdtR = cp.tile([1, GT], F32, tag="sdtR")
            bt_sb = cp.tile([128, GT], F32, tag="bt_sb")
            wrow = cp.tile([1, GT], F32, tag="wrow")
            sdt_bc = cp.tile([128, GT], F32, tag="sdt_bc")
            dt_bc = cp.tile([128, GT], F32, tag="dt_bc")
            w_bc = cp.tile([128, GT], F32, tag="w_bc")
            aneg = cp.tile([128, HC], F32, tag="aneg")
            yg = cp.tile([128, HC * gpc], F32, tag="yg")
            upoolc = cp.tile([128, HC * gpc], F32, tag="upoolc")

            def step_xz(mc):
                p = pe_tail(
                    [win_sb[:, kc * 2 * h + mc * 128:
                            kc * 2 * h + mc * 128 + 128] for kc in range(HC)],
                    [seq_cc(kc) for kc in range(HC)], GT)
                nc.scalar.copy(xzT[:, mc * GT:(mc + 1) * GT], p[:])

            def step_dbc():
                p = pe_tail(
                    [wdt_sb[:, kc * DC:kc * DC + 128] for kc in range(HC)],
                    [seq_cc(kc) for kc in range(HC)], GT)
                nc.scalar.copy(dbc0[:], p[:])
                nc.sync.dma_start(bt_sb[0:s, :], dbc0[s:2 * s, :])

            def step_dt():
                # softplus(x + dt_bias) = ln(1 + exp(x + dt_bias))
                p = pe_tail(
                    [wdt_sb[:, kc * DC + 128:kc * DC + DC]
                     for kc in range(HC)],
                    [seq_cc(kc) for kc in range(HC)], GT, mrows=DC - 128)
                nc.scalar.activation(dtsp[:], p[0:1, :],
                                     mybir.ActivationFunctionType.Exp,
                                     bias=dtb[0:1, 0:1])
                nc.vector.tensor_scalar_add(dtsp[:], dtsp[:], 1.0)
                nc.scalar.activation(dtsp[:], dtsp[:],
                                     mybir.ActivationFunctionType.Ln)

            def step_sdt():
                # suffix sum of dt within each graph
                nc.sync.dma_start(dt_scratch[:], dtsp[0:1, :])
                nc.sync.dma_start(
                    dt2[:T, :gpc],
                    dt_scratch[:].rearrange("(b t) -> t b", b=gpc))
                pS = pt.tile([128, gpc], F32, tag="tp")
                nc.tensor.matmul(pS[:T, :], lhsT=t1c[:T, :T],
                                 rhs=dt2[:T, :gpc], start=True, stop=True)
                nc.scalar.copy(sdt2[:T, :], pS[:T, :])
                nc.sync.dma_start(
                    sdt_scratch[:].rearrange("(b t) -> t b", b=gpc),
                    sdt2[:T, :gpc])
                nc.sync.dma_start(sdtR[0:1, :], sdt_scratch[:])

            def step_wrow():
                wps = pt.tile([1, GT], F32, tag="tp")
                for g in range(gpc):
                    nc.tensor.matmul(
                        wps[0:1, g * T:(g + 1) * T],
                        lhsT=dbc0[0:s, g * T + T - 1:g * T + T],
                        rhs=bt_sb[0:s, g * T:(g + 1) * T],
                        start=True, stop=True)
                nc.scalar.copy(wrow[:], wps[:])

            def bcast_into(row, t):
                p = pt.tile([128, GT], F32, tag="tp")
                nc.tensor.matmul(p[:], lhsT=ones1[0:1, :128],
                                 rhs=row[0:1, :], start=True, stop=True)
                nc.scalar.copy(t[:], p[:])

            def step_aneg():
                nc.scalar.activation(aneg[:], alog[:, :HC],
                                     mybir.ActivationFunctionType.Exp)
                nc.vector.tensor_scalar_mul(aneg[:], aneg[:], -1.0)

            def step_v(cc):
                ge = wp.tile([128, GT], F32, tag="ge")
                nc.vector.tensor_tensor(
                    out=ge[:], in0=sdt_bc[:],
                    in1=aneg[:, cc:cc + 1].to_broadcast([128, GT]),
                    op=mybir.AluOpType.mult)
                nc.scalar.activation(ge[:], ge[:],
                                     mybir.ActivationFunctionType.Exp)
                xcc = xzT[:, cc * GT:(cc + 1) * GT]
                dx = wp.tile([128, GT], F32, tag="dx")
                nc.vector.tensor_tensor(out=dx[:], in0=dt_bc[:], in1=xcc,
                                        op=mybir.AluOpType.mult)
                nc.vector.tensor_tensor(out=ge[:], in0=ge[:], in1=dx[:],
                                        op=mybir.AluOpType.mult)
                nc.vector.tensor_tensor(out=ge[:], in0=ge[:], in1=w_bc[:],
                                        op=mybir.AluOpType.mult)
                ys = wp.tile([128, gpc], F32, tag="ys")
                nc.vector.tensor_reduce(
                    ys[:], ge[:].rearrange("p (b t) -> p b t", b=gpc),
                    axis=mybir.AxisListType.X, op=mybir.AluOpType.add)
                # + Dp * x_last
                xl = xcc.rearrange("p (b t) -> p b t", b=gpc)[:, :, T - 1]
                dpx = wp.tile([128, gpc], F32, tag="dpx")
                nc.vector.tensor_tensor(
                    out=dpx[:], in0=xl,
                    in1=dpc[:, cc:cc + 1].to_broadcast([128, gpc]),
                    op=mybir.AluOpType.mult)
                nc.vector.tensor_add(ys[:], ys[:], dpx[:])
                # gate with silu(z_last)
                zl = xzT[:, (HC + cc) * GT:(HC + cc + 1) * GT].rearrange(
                    "p (b t) -> p b t", b=gpc)[:, :, T - 1]
                sl = wp.tile([128, gpc], F32, tag="sl")
                nc.scalar.activation(sl[:], zl,
                                     mybir.ActivationFunctionType.Sigmoid)
                nc.vector.tensor_tensor(out=sl[:], in0=sl[:], in1=zl,
                                        op=mybir.AluOpType.mult)
                nc.vector.tensor_tensor(
                    out=yg[:, cc * gpc:(cc + 1) * gpc], in0=ys[:], in1=sl[:],
                    op=mybir.AluOpType.mult)

            def step_upool(mc):
                # micro pool^T = (yg @ W_out)^T + u_last
                p = pe_tail(
                    [wout_sb[:, kc * h + mc * 128:kc * h + mc * 128 + 128]
                     for kc in range(HC)],
                    [yg[:, kc * gpc:(kc + 1) * gpc] for kc in range(HC)],
                    gpc)
                ul = seq_cc(mc).rearrange(
                    "p (b t) -> p b t", b=gpc)[:, :, T - 1]
                nc.vector.tensor_tensor(
                    out=upoolc[:, mc * gpc:(mc + 1) * gpc], in0=p[:], in1=ul,
                    op=mybir.AluOpType.add)

            # tail steps, in dependency order; emitted into the gaps of the
            # macro branch's DMA-bound loop
            steps = [lambda g=g: step_seq(g) for g in range(gpc)]
            steps += [lambda mc=mc: step_xz(mc) for mc in range(2 * HC)]
            steps += [step_dbc, step_dt, step_sdt, step_wrow]
            steps += [lambda: bcast_into(sdtR, sdt_bc),
                      lambda: bcast_into(dtsp, dt_bc),
                      lambda: bcast_into(wrow, w_bc), step_aneg]
            steps += [lambda cc=cc: step_v(cc) for cc in range(HC)]
            steps += [lambda mc=mc: step_upool(mc) for mc in range(HC)]

            gcn_mac = gcn_branch(
                "a", D["xs_mac"], D["deg_mac"], D["dl_mac"], D["ew_mac"],
                D["degd_mac"], meta["Tpos_mac"], NWa, Wmac, TA, wgmac, 1,
                co_steps=steps)

            # ---- macro pool^T [h, gpc]
            mpoolc = cp.tile([128, HC * gpc], F32, tag="mpoolc")
            for cc in range(HC):
                pp = pt.tile([128, gpc], F32, tag="tp")
                for g in range(gpc):
                    nc.tensor.matmul(
                        pp[:],
                        lhsT=gcn_mac[:, g * h + cc * 128:
                                     g * h + cc * 128 + 128],
                        rhs=poolm[:, g * gpc:(g + 1) * gpc],
                        start=(g == 0), stop=(g == gpc - 1))
                nc.scalar.activation(
                    mpoolc[:, cc * gpc:(cc + 1) * gpc], pp[:],
                    mybir.ActivationFunctionType.Identity,
                    bias=bgc[:, cc:cc + 1])

            # ---- final MLP
            poolcat = [mpoolc[:, cc * gpc:(cc + 1) * gpc] for cc in range(HC)]
            poolcat += [upoolc[:, cc * gpc:(cc + 1) * gpc] for cc in range(HC)]
            z1 = cp.tile([128, HC * gpc], F32, tag="z1")
            for mc in range(HC):
                p = pe_tail(
                    [w1_sb[:, kc * h + mc * 128:kc * h + mc * 128 + 128]
                     for kc in range(2 * HC)],
                    poolcat, gpc)
                nc.scalar.activation(
                    z1[:, mc * gpc:(mc + 1) * gpc], p[:],
                    mybir.ActivationFunctionType.Relu,
                    bias=b1c[:, mc:mc + 1])
            for mc in range(2 * HC):
                p = pe_tail(
                    [w2_sb[:, kc * 2 * h + mc * 128:
                           kc * 2 * h + mc * 128 + 128] for kc in range(HC)],
                    [z1[:, kc * gpc:(kc + 1) * gpc] for kc in range(HC)],
                    gpc)
                ot = wp.tile([128, gpc], F32, tag="ot")
                nc.scalar.activation(ot[:], p[:],
                                     mybir.ActivationFunctionType.Identity,
                                     bias=b2c[:, mc:mc + 1])
                nc.sync.dma_start(outT[mc * 128:(mc + 1) * 128, :], ot[:])
    nc.compile()
    return nc


# ---------------------------------------------------------------- entry

def kernel(**inputs) -> np.ndarray:
    cfg = REAL
    in_maps, meta = prep_host(inputs, cfg)
    nc = build_nc(cfg, meta)
    res = bass_utils.run_bass_kernel_spmd(
        nc, in_maps, core_ids=list(range(cfg.n_cores)))
    out = np.concatenate([r["outT"].T for r in res.results], axis=0)
    return out[meta["gmap"]].astype(np.float32)



# revision 16
# speedup vs baseline: 1.6280x; 1.6280x over previous
"""Trainium2 Bass kernel for nn_DGSL_3453153706625 (gnn_message_passing).

Strategy (data-parallel over graphs, 8 graphs per core):
  * Only the nodes referenced by gather_idx matter for the micro GCN output
    (<=250 unique per graph), and only the final timestep of the Mamba scan
    feeds the output.  Per graph we build 256 dst "slots" (2 windows of 128)
    and extract the edges whose dst is in the slot set (+1 self edge/slot).
  * Host does index/layout prep only: per-core edge-major bf16 x-slabs
    (subgraph feature extraction), per-edge src-degree weight lists (padded),
    dst-local indices, edge weights.  All FLOPs run on device.
  * Aggregate-first GCN: because GCNConv is linear, raw 384-dim features are
    scattered per window first (one-hot S matmuls over edge tiles, norm
    folded into S on DVE), then one 384->256 transform per 128-slot window:
      deg = rowsum(list); dinv = sqrt(1/deg); S = (iota==dl) * (dinv_src*ew)
      aggXT[feat,slot] += xs_tile^T-free scatter matmuls (bf16, fp32 PSUM)
      gcn[slot,h] = (aggXT^T @ Wg) * dinv_dst   (per window)
    Mamba last-state algebra + macro GCN + mean pools + final MLP as before.
    Output [2H, B/core]^T per core.
"""

import math
from dataclasses import dataclass

import ml_dtypes
import numpy as np

import concourse.bass as bass
import concourse.tile as tile
from concourse import bacc
from concourse import mybir
from concourse import bass_utils

F32 = mybir.dt.float32
BF16 = mybir.dt.bfloat16
BF16NP = ml_dtypes.bfloat16


@dataclass
class Cfg:
    n_cores: int = 8
    gpc: int = 8            # graphs per core
    T: int = 50             # seq len
    NG: int = 5             # nodes per group
    n_micro: int = 131072
    e_micro: int = 1048576
    n_macro: int = 6400
    e_macro: int = 51200
    npm: int = 100          # nodes per macro graph
    in_dim: int = 384
    h: int = 256
    s: int = 64
    chunk_tiles: int = 16   # x-slab DMA chunk, in 128-edge tiles

    @property
    def B(self):
        return self.n_cores * self.gpc

    @property
    def KC(self):
        return self.in_dim // 128

    @property
    def HC(self):
        return self.h // 128


REAL = Cfg()


# ---------------------------------------------------------------- host prep

def _csr_by_dst(dst, ew, n_nodes):
    order = np.argsort(dst, kind="stable")
    counts = np.bincount(dst, minlength=n_nodes).astype(np.int64)
    offs = np.concatenate([[0], np.cumsum(counts)])[:-1]
    return counts, offs, ew[order]


def _deg_lists(node_ids, counts, offs, csr_ew, W):
    """[M, W] padded incoming-edge-weight lists with the +1.0 self entry."""
    node_ids = np.asarray(node_ids, dtype=np.int64)
    M = len(node_ids)
    cnts = counts[node_ids]
    pos = offs[node_ids][:, None] + np.arange(W)[None, :]
    pos = np.minimum(pos, max(len(csr_ew) - 1, 0))
    valid = np.arange(W)[None, :] < cnts[:, None]
    out = np.where(valid, csr_ew[pos], 0.0).astype(np.float32)
    out[np.arange(M), cnts] = 1.0  # self-loop +1
    return out


def _tile_layout_rows(arr_2d, tiles, width):
    """[tiles*128, W] -> [128, tiles*W] partition-line layout."""
    a = arr_2d.reshape(tiles, 128, width).transpose(1, 0, 2)
    return np.ascontiguousarray(a.reshape(128, tiles * width))


def _col_layout(arr_1d, tiles):
    """[tiles*128] -> [128, tiles]."""
    return np.ascontiguousarray(arr_1d.reshape(tiles, 128).T)


def _prep_branch(x, src_all, dst_all, ew_all, n_nodes, slot_nodes, cfg,
                 n_windows_per_graph, gmap=None):
    """Shared micro/macro edge-extraction.

    slot_nodes: list of B arrays (sorted node ids per graph's slots).
    Returns dict with per-core slabs and shared meta.
    """
    B, gpc, ncores = cfg.B, cfg.gpc, cfg.n_cores
    nwg = n_windows_per_graph
    counts, offs, csr_ew = _csr_by_dst(dst_all, ew_all, n_nodes)
    W = int(counts.max()) + 1
    W = int(math.ceil(W / 4) * 4)

    # node -> (graph, local) multimap
    n_g = np.array([len(u) for u in slot_nodes])
    cat_nodes = np.concatenate(slot_nodes)
    cat_graph = np.repeat(np.arange(B), n_g)
    cat_local = np.concatenate([np.arange(n) for n in n_g])
    ordn = np.argsort(cat_nodes, kind="stable")
    snodes = cat_nodes[ordn]

    le = np.searchsorted(snodes, dst_all, "left")
    ri = np.searchsorted(snodes, dst_all, "right")
    cnt = ri - le
    sel = np.flatnonzero(cnt)
    c = cnt[sel]
    rep = np.repeat(sel, c)
    startrep = np.repeat(le[sel], c)
    within = np.arange(int(c.sum())) - np.repeat(np.cumsum(c) - c, c)
    matchpos = ordn[startrep + within]

    e_graph = np.concatenate([cat_graph[matchpos], cat_graph])
    e_local = np.concatenate([cat_local[matchpos], cat_local])
    e_src = np.concatenate([src_all[rep], cat_nodes])
    e_ew = np.concatenate([ew_all[rep], np.ones(len(cat_nodes), np.float32)])

    e_win = e_local // 128
    e_dl = (e_local % 128).astype(np.float32)
    if gmap is None:
        # balance: assign graphs to (core, gpos) so that same-gpos graphs
        # across cores have similar edge counts (cuts the cross-core max
        # padding); sorted-rank round-robin.
        counts_g = np.bincount(e_graph, minlength=B)
        rank = np.argsort(-counts_g, kind="stable")
        gmap = np.empty(B, np.int64)
        for r, g in enumerate(rank):
            gmap[g] = (r % ncores) * gpc + (r // ncores)
    e_slot = gmap[e_graph]
    key = e_slot * nwg + e_win
    counts_gw = np.bincount(key, minlength=B * nwg)
    tiles_gw = np.ceil(counts_gw / 128).astype(np.int64)
    tiles_gw = np.maximum(tiles_gw, 1)
    Tpos = tiles_gw.reshape(ncores, gpc * nwg).max(axis=0)  # [gpc*nwg]
    pos_off = np.concatenate([[0], np.cumsum(Tpos * 128)])
    EM = int(pos_off[-1])

    orde = np.argsort(key, kind="stable")
    segoff = np.concatenate([[0], np.cumsum(counts_gw)])

    srcs = np.zeros((ncores, EM), np.int64)
    ews = np.zeros((ncores, EM), np.float32)
    dloc = np.full((ncores, EM), -1.0, np.float32)
    for g in range(B):
        slot = int(gmap[g])
        core, gpos = slot // gpc, slot % gpc
        for w in range(nwg):
            k = slot * nwg + w
            ck = int(counts_gw[k])
            sl = orde[segoff[k]:segoff[k] + ck]
            o = int(pos_off[gpos * nwg + w])
            srcs[core, o:o + ck] = e_src[sl]
            ews[core, o:o + ck] = e_ew[sl]
            dloc[core, o:o + ck] = e_dl[sl]

    tiles = EM // 128
    per_core = []
    for core in range(ncores):
        # edge-major bf16 slab: [tiles, 128, in_dim]
        xs = x[srcs[core]].astype(BF16NP).reshape(tiles, 128, x.shape[1])
        deg = _deg_lists(srcs[core], counts, offs, csr_ew, W)
        per_core.append(dict(
            xs=np.ascontiguousarray(xs),
            deg=_tile_layout_rows(deg, tiles, W).astype(BF16NP),
            dl=_col_layout(dloc[core], tiles),
            ew=_col_layout(ews[core], tiles),
        ))

    # dst-slot degree lists: [ncores][128, nW*W]
    nW = gpc * nwg
    inv = np.empty(B, np.int64)
    inv[gmap] = np.arange(B)
    for core in range(ncores):
        slot_ids = np.zeros((nW, 128), np.int64)
        for gpos in range(gpc):
            g = int(inv[core * gpc + gpos])
            u = slot_nodes[g]
            for w in range(nwg):
                seg = u[w * 128:(w + 1) * 128]
                slot_ids[gpos * nwg + w, :len(seg)] = seg
        degd = _deg_lists(slot_ids.ravel(), counts, offs, csr_ew, W)
        per_core[core]["degd"] = _tile_layout_rows(degd, nW, W).astype(BF16NP)

    return dict(per_core=per_core, Tpos=Tpos, EM=EM, W=W, gmap=gmap)


def prep_host(inputs, cfg):
    gi = np.asarray(inputs["gather_idx"]).astype(np.int64)  # [B, T, NG]
    mask = np.asarray(inputs["mask"]).astype(np.float32)    # [B, T]
    B, gpc, T, NG = cfg.B, cfg.gpc, cfg.T, cfg.NG

    uniq = [np.unique(gi[g]) for g in range(B)]
    for u in uniq:
        assert len(u) <= 256
    mic = _prep_branch(
        np.asarray(inputs["micro_x"]),
        np.asarray(inputs["micro_ei"][0]).astype(np.int64),
        np.asarray(inputs["micro_ei"][1]).astype(np.int64),
        np.asarray(inputs["micro_ew"]).astype(np.float32),
        cfg.n_micro, uniq, cfg, 2)

    gmap = mic["gmap"]
    mac_slots = [np.arange(g * cfg.npm, (g + 1) * cfg.npm) for g in range(B)]
    mac = _prep_branch(
        np.asarray(inputs["macro_x"]),
        np.asarray(inputs["macro_ei"][0]).astype(np.int64),
        np.asarray(inputs["macro_ei"][1]).astype(np.int64),
        np.asarray(inputs["macro_ew"]).astype(np.float32),
        cfg.n_macro, mac_slots, cfg, 1, gmap=gmap)

    # G slab (mask/NG at (slot, t)) and mask rows, per core
    NWm = gpc * 2
    Gall = np.zeros((cfg.n_cores, NWm, 128, T), np.float32)
    g_idx = np.repeat(np.arange(B), T * NG)
    t_idx = np.tile(np.repeat(np.arange(T), NG), B)
    loc = np.concatenate(
        [np.searchsorted(uniq[g], gi[g].ravel()) for g in range(B)])
    slot_i = gmap[g_idx]
    core_i = slot_i // gpc
    win_i = (slot_i % gpc) * 2 + loc // 128
    row_i = loc % 128
    val = mask[g_idx, t_idx] / NG
    np.add.at(Gall, (core_i, win_i, row_i, t_idx), val)

    # consts
    iota_bf = np.tile(np.arange(128, dtype=np.float32)[None, :],
                      (128, 1)).astype(BF16NP)
    T1 = np.zeros((128, T), np.float32)
    tt = np.arange(T)
    T1[:T, :] = (tt[:, None] > tt[None, :]).astype(np.float32)  # [tau, t]
    ones1 = np.ones((1, 128), np.float32)
    poolmat = np.zeros((128, gpc * gpc), np.float32)
    for g in range(gpc):
        poolmat[:cfg.npm, g * gpc + g] = 1.0 / cfg.npm

    wdt = np.asarray(inputs["W_dtBC"]).astype(np.float32)  # [h, 1+2s]
    s = cfg.s
    wdt_perm = np.concatenate(
        [wdt[:, 1 + s:1 + 2 * s], wdt[:, 1:1 + s], wdt[:, :1]], axis=1)

    f32 = np.float32
    shared = {
        "Wg_mic": np.ascontiguousarray(
            np.asarray(inputs["Wg_micro"]).astype(BF16NP)),
        "Wg_mac": np.ascontiguousarray(
            np.asarray(inputs["Wg_macro"]).astype(BF16NP)),
        "bgm_row": np.asarray(inputs["bg_micro"]).astype(
            BF16NP).reshape(1, -1),
        "bgcT": np.asarray(inputs["bg_macro"]).astype(f32).reshape(-1, 1),
        "W_in": np.asarray(inputs["W_in"]).astype(BF16NP),
        "WdtP": np.ascontiguousarray(wdt_perm).astype(BF16NP),
        "dtb": np.asarray(inputs["dt_bias"]).astype(f32).reshape(1, 1),
        "A_logT": np.asarray(inputs["A_log"]).astype(f32).reshape(-1, 1),
        "DpT": np.asarray(inputs["Dp"]).astype(f32).reshape(-1, 1),
        "W_out": np.asarray(inputs["W_out"]).astype(BF16NP),
        "W1": np.asarray(inputs["W1"]).astype(BF16NP),
        "b1T": np.asarray(inputs["b1"]).astype(f32).reshape(-1, 1),
        "W2": np.asarray(inputs["W2"]).astype(BF16NP),
        "b2T": np.asarray(inputs["b2"]).astype(f32).reshape(-1, 1),
        "iota_bf": iota_bf, "ones1": ones1,
        "poolmat": poolmat.astype(BF16NP),
    }

    inv_g = np.empty(B, np.int64)
    inv_g[gmap] = np.arange(B)
    in_maps = []
    for core in range(cfg.n_cores):
        m = dict(shared)
        pc, qc = mic["per_core"][core], mac["per_core"][core]
        m.update({
            "xs_mic": pc["xs"], "deg_mic": pc["deg"], "dl_mic": pc["dl"],
            "ew_mic": pc["ew"], "degd_mic": pc["degd"],
            "xs_mac": qc["xs"], "deg_mac": qc["deg"], "dl_mac": qc["dl"],
            "ew_mac": qc["ew"], "degd_mac": qc["degd"],
            "Gslab": np.ascontiguousarray(
                Gall[core].transpose(1, 0, 2).reshape(
                    128, NWm * T)).astype(BF16NP),
            "maskrow": np.ascontiguousarray(
                mask[inv_g[core * gpc:(core + 1) * gpc]].reshape(
                    1, gpc * T)).astype(BF16NP),
        })
        in_maps.append(m)

    meta = dict(
        Tpos_mic=mic["Tpos"], EM=mic["EM"], Wmic=mic["W"],
        Tpos_mac=mac["Tpos"], EA=mac["EM"], Wmac=mac["W"],
        gmap=gmap,
    )
    return in_maps, meta


# ---------------------------------------------------------------- device

def build_nc(cfg, meta):
    T, gpc, h, s = cfg.T, cfg.gpc, cfg.h, cfg.s
    KC, HC = cfg.KC, cfg.HC
    DC = 1 + 2 * s
    IND = cfg.in_dim
    assert 2 * s <= 128 and T <= 128 and gpc * T <= 512
    EM, EA = meta["EM"], meta["EA"]
    Wmic, Wmac = meta["Wmic"], meta["Wmac"]
    NWm, NWa = gpc * 2, gpc
    TM, TA = EM // 128, EA // 128

    nc = bacc.Bacc("TRN2")
    D = {}
    def din(name, shape, dt=F32):
        D[name] = nc.dram_tensor(name, list(shape), dt, kind="ExternalInput")
        return D[name]

    din("xs_mic", (TM, 128, IND), BF16)
    din("deg_mic", (128, TM * Wmic), BF16)
    din("dl_mic", (128, TM))
    din("ew_mic", (128, TM))
    din("degd_mic", (128, NWm * Wmic), BF16)
    din("xs_mac", (TA, 128, IND), BF16)
    din("deg_mac", (128, TA * Wmac), BF16)
    din("dl_mac", (128, TA))
    din("ew_mac", (128, TA))
    din("degd_mac", (128, NWa * Wmac), BF16)
    din("Gslab", (128, NWm * T), BF16)
    din("maskrow", (1, gpc * T), BF16)
    din("Wg_mic", (IND, h), BF16)
    din("Wg_mac", (IND, h), BF16)
    din("bgm_row", (1, h), BF16)
    din("bgcT", (h, 1))
    din("W_in", (h, 2 * h), BF16)
    din("WdtP", (h, DC), BF16)
    din("dtb", (1, 1))
    din("A_logT", (h, 1))
    din("DpT", (h, 1))
    din("W_out", (h, h), BF16)
    din("W1", (2 * h, h), BF16)
    din("b1T", (h, 1))
    din("W2", (h, 2 * h), BF16)
    din("b2T", (2 * h, 1))
    din("iota_bf", (128, 128), BF16)
    din("ones1", (1, 128))
    din("poolmat", (128, gpc * gpc), BF16)
    outT = nc.dram_tensor("outT", [2 * h, gpc], F32, kind="ExternalOutput")

    with tile.TileContext(nc) as tc:
        with (
            tc.tile_pool(name="const", bufs=1) as cp,
            tc.tile_pool(name="xs", bufs=3) as xp,
            tc.tile_pool(name="degs", bufs=4) as dp,
            tc.tile_pool(name="work", bufs=8) as wp,
            tc.tile_pool(name="ph", bufs=2, space="PSUM") as ph,
            tc.tile_pool(name="pagg", bufs=2, space="PSUM") as pagg,
            tc.tile_pool(name="ptail", bufs=2, space="PSUM") as pt,
        ):
            def pe_touch(ap_col):
                """Dummy weight-load so PE's vector clock absorbs the DMA
                wait of an operand before its real (1-wait-budget)
                matmul.  No PSUM output, single LW struct, single wait."""
                nc.tensor.ldweights(ap_col.bitcast(BF16))
            def load_const(name):
                src = D[name]
                t = cp.tile(list(src.shape), src.dtype, tag=name)
                nc.sync.dma_start(t[:], src[:])
                return t

            def load_mat_chunks(name, k, n, dt=F32):
                """[k, n] dram -> SBUF [128, (k//128)*n], chunk kc at
                cols [kc*n:(kc+1)*n].  Single DMA."""
                kc_n = k // 128
                t = cp.tile([128, kc_n * n], dt, tag=name)
                nc.sync.dma_start(
                    t[:].rearrange("p (c n) -> p c n", c=kc_n),
                    D[name][:].rearrange("(c p) n -> p c n", p=128))
                return t

            wgmic = load_mat_chunks("Wg_mic", IND, h, BF16)
            wgmac = load_mat_chunks("Wg_mac", IND, h, BF16)
            for kc in range(KC):
                pe_touch(wgmic[:, kc * h:kc * h + 1])
                pe_touch(wgmac[:, kc * h:kc * h + 1])
            iota = load_const("iota_bf")

            def gcn_branch(tag, xs_d, deg_d, dl_d, ew_d, degd_d, Tpos, nwin,
                           Wd, ntiles, wg_sb, co_steps=None):
                # dst dinv per window
                degd_sb = cp.tile([128, nwin * Wd], BF16, tag=f"degd{tag}")
                nc.scalar.dma_start(degd_sb[:], degd_d[:])
                dsum = cp.tile([128, nwin], F32, tag=f"dsum{tag}")
                nc.vector.tensor_reduce(
                    dsum[:], degd_sb[:].rearrange("p (w d) -> p w d", d=Wd),
                    axis=mybir.AxisListType.X, op=mybir.AluOpType.add)
                nc.vector.reciprocal(dsum[:], dsum[:])
                dinvd = cp.tile([128, nwin], F32, tag=f"dinvd{tag}")
                nc.scalar.sqrt(dinvd[:], dsum[:])

                # per-edge scal = rsqrt(deg_src)*ew for ALL tiles, batched up
                # front (one Rsqrt total -> no ACT table thrash in the loop)
                deg_all = cp.tile([128, ntiles * Wd], BF16, tag=f"dega{tag}")
                nc.scalar.dma_start(deg_all[:], deg_d[:])
                scal_all = cp.tile([128, ntiles], F32, tag=f"scala{tag}")
                RB = 48
                for r0 in range(0, ntiles, RB):
                    rt = min(RB, ntiles - r0)
                    nc.vector.tensor_reduce(
                        scal_all[:, r0:r0 + rt],
                        deg_all[:, r0 * Wd:(r0 + rt) * Wd].rearrange(
                            "p (t d) -> p t d", d=Wd),
                        axis=mybir.AxisListType.X, op=mybir.AluOpType.add)
                nc.vector.reciprocal(scal_all[:], scal_all[:])
                nc.scalar.sqrt(scal_all[:], scal_all[:])

                dl_sb = cp.tile([128, ntiles], F32, tag=f"dl{tag}")
                nc.scalar.dma_start(dl_sb[:], dl_d[:])
                ew_sb = cp.tile([128, ntiles], F32, tag=f"ew{tag}")
                nc.scalar.dma_start(ew_sb[:], ew_d[:])
                nc.vector.tensor_tensor(
                    out=scal_all[:], in0=scal_all[:], in1=ew_sb[:],
                    op=mybir.AluOpType.mult)

                gcnw = cp.tile([128, nwin * h], BF16, tag=f"gcnw{tag}")

                # tile -> window map
                win_of, idx_in, len_of = [], [], []
                for p, tp in enumerate(Tpos):
                    for i in range(int(tp)):
                        win_of.append(p)
                        idx_in.append(i)
                        len_of.append(int(tp))

                def emit_transform(w, aggsb):
                    outp = ph.tile([128, h], F32, tag="hp")
                    for kc in range(KC):
                        nc.tensor.matmul(
                            outp[:],
                            lhsT=aggsb[:, kc * 128:(kc + 1) * 128],
                            rhs=wg_sb[:, kc * h:(kc + 1) * h],
                            start=(kc == 0), stop=(kc == KC - 1))
                    nc.scalar.mul(
                        gcnw[:, w * h:(w + 1) * h], outp[:],
                        dinvd[:, w:w + 1])

                CT = cfg.chunk_tiles
                agg = None
                pending = None
                nch = (ntiles + CT - 1) // CT
                co_done = 0
                for c0 in range(0, ntiles, CT):
                    ct = min(CT, ntiles - c0)
                    xt = xp.tile([128, CT * IND], BF16, tag="xt")
                    nc.sync.dma_start(
                        xt[:, :ct * IND].rearrange("p (t f) -> p t f", t=ct),
                        xs_d[c0:c0 + ct].rearrange("t p f -> p t f"))
                    pe_touch(xt[:, 0:1])

                    for i in range(ct):
                        ti = c0 + i
                        S = wp.tile([128, 128], BF16, tag="S0")
                        nc.vector.tensor_scalar(
                            S[:], iota[:], dl_sb[:, ti:ti + 1],
                            scal_all[:, ti:ti + 1],
                            mybir.AluOpType.is_equal, mybir.AluOpType.mult)
                        if idx_in[ti] == 0:
                            agg = pagg.tile([128, KC * 128], F32, tag="agg")
                        last = idx_in[ti] == len_of[ti] - 1
                        for kc in range(KC):
                            nc.tensor.matmul(
                                agg[:, kc * 128:(kc + 1) * 128],
                                lhsT=xt[:, i * IND + kc * 128:
                                        i * IND + (kc + 1) * 128],
                                rhs=S[:],
                                start=(idx_in[ti] == 0 and kc == 0),
                                stop=last)
                        if last:
                            # evacuate now; defer the transform one window so
                            # the ACT evac latency hides behind the next
                            # window's scatter matmuls
                            aggsb = wp.tile([128, KC * 128], BF16,
                                            tag="aggsb")
                            nc.scalar.copy(aggsb[:], agg[:])
                            if pending is not None:
                                emit_transform(*pending)
                            pending = (win_of[ti], aggsb)
                    if co_steps is not None:
                        want = (len(co_steps) * (c0 // CT + 1)) // max(
                            1, nch - 1)
                        want = min(want, len(co_steps))
                        while co_done < want:
                            co_steps[co_done]()
                            co_done += 1
                if pending is not None:
                    emit_transform(*pending)
                if co_steps is not None:
                    while co_done < len(co_steps):
                        co_steps[co_done]()
                        co_done += 1
                return gcnw

            gcn_mic = gcn_branch(
                "m", D["xs_mic"], D["deg_mic"], D["dl_mic"], D["ew_mic"],
                D["degd_mic"], meta["Tpos_mic"], NWm, Wmic, TM, wgmic)

            ones1 = load_const("ones1")
            poolm = load_const("poolmat")
            gsl = load_const("Gslab")
            mrow = load_const("maskrow")
            win_sb = load_mat_chunks("W_in", h, 2 * h, BF16)
            wdt_sb = load_mat_chunks("WdtP", h, DC, BF16)
            wout_sb = load_mat_chunks("W_out", h, h, BF16)
            w1_sb = load_mat_chunks("W1", 2 * h, h, BF16)
            w2_sb = load_mat_chunks("W2", h, 2 * h, BF16)
            bgm = load_const("bgm_row")
            bgc = load_mat_chunks("bgcT", h, 1)
            b1c = load_mat_chunks("b1T", h, 1)
            b2c = load_mat_chunks("b2T", 2 * h, 1)
            alog = load_mat_chunks("A_logT", h, 1)
            dpc = load_mat_chunks("DpT", h, 1)
            dtb = load_const("dtb")
            # ---- tail (seq^T + mamba), emitted as steps interleaved
            # into the macro branch's DMA-bound chunk loop
            GT = gpc * T
            seqT = cp.tile([128, HC * gpc * T], BF16, tag="seqT")

            def seq_cc(cc):
                return seqT[:, cc * gpc * T:(cc + 1) * gpc * T]

            def step_seq(g):
                for cc in range(HC):
                    ps = pt.tile([128, T], F32, tag="tp")
                    nc.tensor.matmul(
                        ps[:], lhsT=bgm[0:1, cc * 128:(cc + 1) * 128],
                        rhs=mrow[0:1, g * T:(g + 1) * T],
                        start=True, stop=False)
                    for w in range(2):
                        wi = g * 2 + w
                        nc.tensor.matmul(
                            ps[:],
                            lhsT=gcn_mic[:, wi * h + cc * 128:
                                         wi * h + cc * 128 + 128],
                            rhs=gsl[:, wi * T:(wi + 1) * T],
                            start=False, stop=(w == 1))
                    nc.scalar.copy(
                        seqT[:, cc * gpc * T + g * T:
                             cc * gpc * T + (g + 1) * T], ps[:])

            def pe_tail(lhsT_list, rhs_list, n, tag="tp", mrows=128):
                p = pt.tile([128, n], F32, tag=tag)
                kn = len(lhsT_list)
                for i, (l, r) in enumerate(zip(lhsT_list, rhs_list)):
                    nc.tensor.matmul(p[:mrows, :], lhsT=l, rhs=r,
                                     start=(i == 0), stop=(i == kn - 1))
                return p

            xzT = cp.tile([128, 4 * GT], F32, tag="xzT")
            dbc0 = cp.tile([128, GT], F32, tag="dbc0")
            dtsp = cp.tile([1, GT], F32, tag="dtsp")
            bt_sb = cp.tile([128, GT], F32, tag="bt_sb")
            wrow = cp.tile([1, GT], F32, tag="wrow")
            sdt_bc = cp.tile([128, GT], F32, tag="sdt_bc")
            dt_bc = cp.tile([128, GT], F32, tag="dt_bc")
            w_bc = cp.tile([128, GT], F32, tag="w_bc")
            aneg = cp.tile([128, HC], F32, tag="aneg")
            yg = cp.tile([128, HC * gpc], BF16, tag="yg")
            upoolc = cp.tile([128, HC * gpc], BF16, tag="upoolc")

            def step_xz(mc):
                p = pe_tail(
                    [win_sb[:, kc * 2 * h + mc * 128:
                            kc * 2 * h + mc * 128 + 128] for kc in range(HC)],
                    [seq_cc(kc) for kc in range(HC)], GT)
                nc.scalar.copy(xzT[:, mc * GT:(mc + 1) * GT], p[:])

            def step_dbc():
                p = pe_tail(
                    [wdt_sb[:, kc * DC:kc * DC + 128] for kc in range(HC)],
                    [seq_cc(kc) for kc in range(HC)], GT)
                nc.scalar.copy(dbc0[:], p[:])
                nc.scalar.dma_start(bt_sb[0:s, :], dbc0[s:2 * s, :])

            def step_dt():
                # softplus(x + dt_bias) = ln(1 + exp(x + dt_bias))
                p = pe_tail(
                    [wdt_sb[:, kc * DC + 128:kc * DC + DC]
                     for kc in range(HC)],
                    [seq_cc(kc) for kc in range(HC)], GT, mrows=DC - 128)
                nc.scalar.activation(dtsp[:], p[0:1, :],
                                     mybir.ActivationFunctionType.Exp,
                                     bias=dtb[0:1, 0:1])
                nc.vector.tensor_scalar_add(dtsp[:], dtsp[:], 1.0)
                nc.scalar.activation(dtsp[:], dtsp[:],
                                     mybir.ActivationFunctionType.Ln)

            def step_sdt():
                # suffix sum of dt within each graph, fully on-device:
                # suffix = total - inclusive_cumsum (per-partition DVE scan)
                cums = cp.tile([128, GT], F32, tag="cums")
                for g in range(gpc):
                    nc.vector.tensor_tensor_scan(
                        cums[:, g * T:(g + 1) * T],
                        dt_bc[:, g * T:(g + 1) * T],
                        dt_bc[:, g * T:(g + 1) * T], 0.0,
                        mybir.AluOpType.add, mybir.AluOpType.bypass)
                tot = cp.tile([128, gpc], F32, tag="tot")
                nc.vector.tensor_reduce(
                    tot[:], dt_bc[:].rearrange("p (b t) -> p b t", b=gpc),
                    axis=mybir.AxisListType.X, op=mybir.AluOpType.add)
                for g in range(gpc):
                    nc.vector.tensor_tensor(
                        out=sdt_bc[:, g * T:(g + 1) * T],
                        in0=tot[:, g:g + 1].to_broadcast([128, T]),
                        in1=cums[:, g * T:(g + 1) * T],
                        op=mybir.AluOpType.subtract)

            def step_wrow():
                wps = pt.tile([1, GT], F32, tag="tp")
                for g in range(gpc):
                    nc.tensor.matmul(
                        wps[0:1, g * T:(g + 1) * T],
                        lhsT=dbc0[0:s, g * T + T - 1:g * T + T],
                        rhs=bt_sb[0:s, g * T:(g + 1) * T],
                        start=True, stop=True)
                nc.scalar.copy(wrow[:], wps[:])

            def bcast_into(row, t):
                p = pt.tile([128, GT], F32, tag="tp")
                nc.tensor.matmul(p[:], lhsT=ones1[0:1, :128],
                                 rhs=row[0:1, :], start=True, stop=True)
                nc.scalar.copy(t[:], p[:])

            def step_aneg():
                nc.scalar.activation(aneg[:], alog[:, :HC],
                                     mybir.ActivationFunctionType.Exp)
                nc.vector.tensor_scalar_mul(aneg[:], aneg[:], -1.0)

            def step_v(cc):
                # decay = exp(sdt * A_cc): A folded in as per-partition scale
                ge = wp.tile([128, GT], F32, tag="ge")
                nc.scalar.activation(ge[:], sdt_bc[:],
                                     mybir.ActivationFunctionType.Exp,
                                     scale=aneg[:, cc:cc + 1])
                xcc = xzT[:, cc * GT:(cc + 1) * GT]
                dx = wp.tile([128, GT], F32, tag="dx")
                nc.vector.tensor_tensor(out=dx[:], in0=dt_bc[:], in1=xcc,
                                        op=mybir.AluOpType.mult)
                nc.vector.tensor_tensor(out=ge[:], in0=ge[:], in1=dx[:],
                                        op=mybir.AluOpType.mult)
                nc.vector.tensor_tensor(out=ge[:], in0=ge[:], in1=w_bc[:],
                                        op=mybir.AluOpType.mult)
                ys = wp.tile([128, gpc], F32, tag="ys")
                nc.vector.tensor_reduce(
                    ys[:], ge[:].rearrange("p (b t) -> p b t", b=gpc),
                    axis=mybir.AxisListType.X, op=mybir.AluOpType.add)
                # + Dp * x_last
                xl = xcc.rearrange("p (b t) -> p b t", b=gpc)[:, :, T - 1]
                dpx = wp.tile([128, gpc], F32, tag="dpx")
                nc.vector.tensor_tensor(
                    out=dpx[:], in0=xl,
                    in1=dpc[:, cc:cc + 1].to_broadcast([128, gpc]),
                    op=mybir.AluOpType.mult)
                nc.vector.tensor_add(ys[:], ys[:], dpx[:])
                # gate with silu(z_last); sigmoid via exp keeps one ACT set
                zl = xzT[:, (HC + cc) * GT:(HC + cc + 1) * GT].rearrange(
                    "p (b t) -> p b t", b=gpc)[:, :, T - 1]
                sl = wp.tile([128, gpc], F32, tag="sl")
                nc.scalar.activation(sl[:], zl,
                                     mybir.ActivationFunctionType.Exp,
                                     scale=-1.0)
                nc.vector.tensor_scalar_add(sl[:], sl[:], 1.0)
                nc.vector.reciprocal(sl[:], sl[:])
                nc.vector.tensor_tensor(out=sl[:], in0=sl[:], in1=zl,
                                        op=mybir.AluOpType.mult)
                nc.vector.tensor_tensor(
                    out=yg[:, cc * gpc:(cc + 1) * gpc], in0=ys[:], in1=sl[:],
                    op=mybir.AluOpType.mult)

            def step_upool(mc):
                # micro pool^T = (yg @ W_out)^T + u_last
                p = pe_tail(
                    [wout_sb[:, kc * h + mc * 128:kc * h + mc * 128 + 128]
                     for kc in range(HC)],
                    [yg[:, kc * gpc:(kc + 1) * gpc] for kc in range(HC)],
                    gpc)
                ul = seq_cc(mc).rearrange(
                    "p (b t) -> p b t", b=gpc)[:, :, T - 1]
                nc.vector.tensor_tensor(
                    out=upoolc[:, mc * gpc:(mc + 1) * gpc], in0=p[:], in1=ul,
                    op=mybir.AluOpType.add)

            # tail steps, in dependency order; emitted into the gaps of the
            # macro branch's DMA-bound loop
            steps = [lambda g=g: step_seq(g) for g in range(gpc)]
            steps += [lambda mc=mc: step_xz(mc) for mc in range(2 * HC)]
            steps += [step_dbc, step_dt,
                      lambda: bcast_into(dtsp, dt_bc), step_sdt, step_wrow]
            steps += [lambda: bcast_into(wrow, w_bc), step_aneg]
            steps += [lambda cc=cc: step_v(cc) for cc in range(HC)]
            steps += [lambda mc=mc: step_upool(mc) for mc in range(HC)]

            gcn_mac = gcn_branch(
                "a", D["xs_mac"], D["deg_mac"], D["dl_mac"], D["ew_mac"],
                D["degd_mac"], meta["Tpos_mac"], NWa, Wmac, TA, wgmac,
                co_steps=steps)

            # ---- macro pool^T [h, gpc]
            mpoolc = cp.tile([128, HC * gpc], BF16, tag="mpoolc")
            for cc in range(HC):
                pp = pt.tile([128, gpc], F32, tag="tp")
                for g in range(gpc):
                    nc.tensor.matmul(
                        pp[:],
                        lhsT=gcn_mac[:, g * h + cc * 128:
                                     g * h + cc * 128 + 128],
                        rhs=poolm[:, g * gpc:(g + 1) * gpc],
                        start=(g == 0), stop=(g == gpc - 1))
                nc.scalar.activation(
                    mpoolc[:, cc * gpc:(cc + 1) * gpc], pp[:],
                    mybir.ActivationFunctionType.Identity,
                    bias=bgc[:, cc:cc + 1])

            # ---- final MLP
            poolcat = [mpoolc[:, cc * gpc:(cc + 1) * gpc] for cc in range(HC)]
            poolcat += [upoolc[:, cc * gpc:(cc + 1) * gpc] for cc in range(HC)]
            z1 = cp.tile([128, HC * gpc], BF16, tag="z1")
            for mc in range(HC):
                p = pe_tail(
                    [w1_sb[:, kc * h + mc * 128:kc * h + mc * 128 + 128]
                     for kc in range(2 * HC)],
                    poolcat, gpc)
                nc.scalar.activation(
                    z1[:, mc * gpc:(mc + 1) * gpc], p[:],
                    mybir.ActivationFunctionType.Relu,
                    bias=b1c[:, mc:mc + 1])
            for mc in range(2 * HC):
                p = pe_tail(
                    [w2_sb[:, kc * 2 * h + mc * 128:
                           kc * 2 * h + mc * 128 + 128] for kc in range(HC)],
                    [z1[:, kc * gpc:(kc + 1) * gpc] for kc in range(HC)],
                    gpc)
                ot = wp.tile([128, gpc], F32, tag="ot")
                nc.scalar.activation(ot[:], p[:],
                                     mybir.ActivationFunctionType.Identity,
                                     bias=b2c[:, mc:mc + 1])
                nc.sync.dma_start(outT[mc * 128:(mc + 1) * 128, :], ot[:])
    nc.compile()
    return nc


# ---------------------------------------------------------------- entry

def kernel(**inputs) -> np.ndarray:
    cfg = REAL
    in_maps, meta = prep_host(inputs, cfg)
    nc = build_nc(cfg, meta)
    res = bass_utils.run_bass_kernel_spmd(
        nc, in_maps, core_ids=list(range(cfg.n_cores)))
    out = np.concatenate([r["outT"].T for r in res.results], axis=0)
    return out[meta["gmap"]].astype(np.float32)
